# revision 8
# baseline (speedup 1.0000x reference)
"""AFT-attention (nn_AFTAttention) distributed Bass kernel for 8 TRN2 NeuronCores.

Reference computation (B=8, T=4096, D=H=1024):
    Q = x @ Wq.T + bq ; K = x @ Wk.T + bk ; V = x @ Wv.T + bv      # [B,T,H]
    numer = exp(K + wbias)                                          # [B,T,H]
    denom = numer.sum(axis=0)                                       # [T,H]
    weighted = (numer * V).sum(axis=0) / denom                      # [T,H]
    out = sigmoid(Q) * weighted                                     # [B,T,H]

Sharding: the reductions are over the BATCH axis only, so sharding T across
the 8 cores makes every reduction core-local -- zero collectives.  Each core
handles all 8 batches for its 512-timestep slice.

Per-core layout: tiles are [h(128 partitions), t(512 free)] so every
per-feature constant (bq, bk+wbias, bv) rides the per-partition scalar port
of ScalarE activation / DVE scalar_tensor_tensor.  Host pre-transposes x to
x^T[b, d, t] and weights to W^T[d, h], and converts to bf16 (matmul runs at
1 col/cycle bf16 vs 4 cycles fp32).  sigmoid is computed via tanh (same ACT
table set as exp -> no 2.7us table reloads):
    sigmoid(q) * w == (tanh(q/2) + 1) * (w/2)
"""

import numpy as np

B, T, D, H = 8, 4096, 1024, 1024
NCORES = 8
TC = T // NCORES      # 512 timesteps per core
P = 128               # partitions
NCH = D // P          # 8 contraction chunks
NHT = H // P          # 8 h tiles
BG = 4                # batch group size for PSUM bank pressure

_cached = None        # (nc, run_fn)


def _build_bass():
    import concourse.bass as bass
    import concourse.mybir as mybir
    import concourse.tile as tile
    from concourse import bacc

    f32 = mybir.dt.float32
    bf16 = mybir.dt.bfloat16
    AF = mybir.ActivationFunctionType
    OP = mybir.AluOpType

    nc = bacc.Bacc(None)

    xt = nc.declare_dram_parameter("xt", [B, NCH, P, TC], bf16, isOutput=False)
    wk = nc.declare_dram_parameter("wk", [NCH, P, H], bf16, isOutput=False)
    wv = nc.declare_dram_parameter("wv", [NCH, P, H], bf16, isOutput=False)
    wq = nc.declare_dram_parameter("wq", [NCH, P, H], bf16, isOutput=False)
    # per-partition constants, host-prearranged as [P, NHT]
    bkw = nc.declare_dram_parameter("bkw", [P, NHT], f32, isOutput=False)  # bk + wbias
    bvp = nc.declare_dram_parameter("bvp", [P, NHT], f32, isOutput=False)  # bv
    bqh = nc.declare_dram_parameter("bqh", [P, NHT], f32, isOutput=False)  # 0.5*bq
    outt = nc.declare_dram_parameter("outt", [B, H, TC], bf16, isOutput=True)

    from contextlib import ExitStack

    with tile.TileContext(nc) as tc, ExitStack() as ctx:
        sing = ctx.enter_context(tc.tile_pool(name="sing", bufs=1))
        ps = ctx.enter_context(tc.tile_pool(name="ps", bufs=8, space="PSUM"))
        acc = ctx.enter_context(tc.tile_pool(name="acc", bufs=2))
        tmp = ctx.enter_context(tc.tile_pool(name="tmp", bufs=6))
        outp = ctx.enter_context(tc.tile_pool(name="outp", bufs=6))

        # --- resident inputs -------------------------------------------------
        # weights: [P, NCH, H] per matrix; lhsT slice = w_sb[:, c, j*P:(j+1)*P]
        wk_sb = sing.tile([P, NCH, H], bf16)
        wv_sb = sing.tile([P, NCH, H], bf16)
        wq_sb = sing.tile([P, NCH, H], bf16)
        # x^T: [P, B, NCH, TC]; rhs slice = xt_sb[:, b, c, :]
        xt_sb = sing.tile([P, B, NCH, TC], bf16)
        bkw_sb = sing.tile([P, NHT], f32)
        bvp_sb = sing.tile([P, NHT], f32)
        bqh_sb = sing.tile([P, NHT], f32)

        nc.sync.dma_start(out=bkw_sb, in_=bkw[:, :])
        nc.sync.dma_start(out=bvp_sb, in_=bvp[:, :])
        nc.sync.dma_start(out=bqh_sb, in_=bqh[:, :])
        # per-chunk DMAs so the first matmuls only wait on what they need
        for c in range(NCH):
            nc.sync.dma_start(out=wk_sb[:, c, :], in_=wk[c])
        for c in range(NCH):
            nc.sync.dma_start(out=wv_sb[:, c, :], in_=wv[c])
        for c in range(NCH):
            for b in range(B):
                nc.sync.dma_start(out=xt_sb[:, b, c, :], in_=xt[b, c])
        for c in range(NCH):
            nc.sync.dma_start(out=wq_sb[:, c, :], in_=wq[c])

        # --- main loop over h tiles -----------------------------------------
        for j in range(NHT):
            hs = bass.ts(j, P)  # h slice in weight free dim

            ns_sb = acc.tile([P, TC], f32, tag="ns")   # sum_b numer
            nv_sb = acc.tile([P, TC], f32, tag="nv")   # sum_b numer * v

            for g in range(B // BG):
                bs = range(g * BG, (g + 1) * BG)
                # K and V matmuls, chunk-outer / batch-inner (shares LDWEIGHTS)
                kps = {b: ps.tile([P, TC], f32, tag="ps", name=f"kps_{j}_{b}") for b in bs}
                vps = {b: ps.tile([P, TC], f32, tag="ps", name=f"vps_{j}_{b}") for b in bs}
                for c in range(NCH):
                    for b in bs:
                        nc.tensor.matmul(
                            kps[b], wk_sb[:, c, hs], xt_sb[:, b, c, :],
                            start=(c == 0), stop=(c == NCH - 1),
                        )
                    for b in bs:
                        nc.tensor.matmul(
                            vps[b], wv_sb[:, c, hs], xt_sb[:, b, c, :],
                            start=(c == 0), stop=(c == NCH - 1),
                        )
                # epilogue: numer = exp(k + bk + wbias); nv += numer*(v + bv)
                for b in bs:
                    if b == 0:
                        numer = ns_sb  # first batch: exp writes the accumulator
                    else:
                        numer = tmp.tile([P, TC], f32, tag="numer", bufs=4)
                    nc.scalar.activation(
                        out=numer, in_=kps[b], func=AF.Exp,
                        bias=bkw_sb[:, j : j + 1], scale=1.0,
                    )
                    if b == 0:
                        nvt = nv_sb
                    else:
                        nvt = tmp.tile([P, TC], f32, tag="nvt", bufs=3)
                    # nvt = (v_psum + bv) * numer
                    nc.vector.scalar_tensor_tensor(
                        out=nvt, in0=vps[b], scalar=bvp_sb[:, j : j + 1],
                        in1=numer, op0=OP.add, op1=OP.mult,
                    )
                    if b != 0:
                        nc.gpsimd.tensor_add(ns_sb, ns_sb, numer)
                        nc.vector.tensor_add(nv_sb, nv_sb, nvt)

            # weighted_half = 0.5 * nv / ns
            rec = tmp.tile([P, TC], f32, tag="rec", bufs=2)
            nc.vector.reciprocal(rec, ns_sb)
            wh = tmp.tile([P, TC], f32, tag="wh", bufs=2)
            nc.vector.scalar_tensor_tensor(
                out=wh, in0=nv_sb, scalar=0.5, in1=rec, op0=OP.mult, op1=OP.mult,
            )

            # Q matmuls + out = (tanh(q/2 + bq/2) + 1) * weighted_half
            for g in range(B // BG):
                bs = range(g * BG, (g + 1) * BG)
                qps = {b: ps.tile([P, TC], f32, tag="ps", name=f"qps_{j}_{b}") for b in bs}
                for c in range(NCH):
                    for b in bs:
                        nc.tensor.matmul(
                            qps[b], wq_sb[:, c, hs], xt_sb[:, b, c, :],
                            start=(c == 0), stop=(c == NCH - 1),
                        )
                for b in bs:
                    th = tmp.tile([P, TC], f32, tag="th", bufs=4)
                    nc.scalar.activation(
                        out=th, in_=qps[b], func=AF.Tanh,
                        bias=bqh_sb[:, j : j + 1], scale=0.5,
                    )
                    ot = outp.tile([P, TC], bf16, tag="ot")
                    nc.vector.scalar_tensor_tensor(
                        out=ot, in0=th, scalar=1.0, in1=wh, op0=OP.add, op1=OP.mult,
                    )
                    nc.sync.dma_start(out=outt[b, hs, :], in_=ot)

    nc.finalize()
    return nc


def _prepare_in_maps(x, Wq, bq, Wk, bk, Wv, bv, wbias):
    import ml_dtypes

    bf16 = ml_dtypes.bfloat16
    f32 = np.float32

    # weights: W.T [D, H] -> [NCH, P, H] bf16 (shared by all cores)
    def prep_w(w):
        return np.ascontiguousarray(w.T.astype(bf16)).reshape(NCH, P, H)

    wq_h = prep_w(np.asarray(Wq))
    wk_h = prep_w(np.asarray(Wk))
    wv_h = prep_w(np.asarray(Wv))

    # per-partition constants as [P, NHT]: col j holds values for h in [j*128, ...)
    def prep_b(v):
        return np.ascontiguousarray(np.asarray(v, f32).reshape(NHT, P).T)

    bkw_h = prep_b(np.asarray(bk, f32) + np.asarray(wbias, f32))
    bvp_h = prep_b(bv)
    bqh_h = prep_b(0.5 * np.asarray(bq, f32))

    x = np.asarray(x)
    in_maps = []
    for core in range(NCORES):
        xs = x[:, core * TC : (core + 1) * TC, :]           # [B, TC, D]
        xtc = np.ascontiguousarray(xs.transpose(0, 2, 1).astype(bf16)).reshape(
            B, NCH, P, TC
        )
        in_maps.append(
            {
                "xt": xtc,
                "wq": wq_h,
                "wk": wk_h,
                "wv": wv_h,
                "bkw": bkw_h,
                "bvp": bvp_h,
                "bqh": bqh_h,
            }
        )
    return in_maps


def _get_nc():
    global _cached
    if _cached is None:
        _cached = _build_bass()
    return _cached


TRACE = False          # set True from a test harness to profile
TRACE_TMPDIR = None    # optional persistent dir for trace artifacts
LAST_RESULT = None     # BassKernelResults of the most recent kernel() call


def kernel(x, Wq, bq, Wk, bk, Wv, bv, wbias):
    global LAST_RESULT
    from concourse.bass_utils import run_bass_kernel_spmd

    nc = _get_nc()
    in_maps = _prepare_in_maps(x, Wq, bq, Wk, bk, Wv, bv, wbias)
    kw = {}
    if TRACE:
        kw = {"trace": True, "tmpdir": TRACE_TMPDIR}
    res = run_bass_kernel_spmd(nc, in_maps, core_ids=list(range(NCORES)), **kw)
    LAST_RESULT = res
    out = np.empty((B, T, H), np.float32)
    for core in range(NCORES):
        o = np.asarray(res.results[core]["outt"])            # [B, H, TC] bf16
        out[:, core * TC : (core + 1) * TC, :] = o.astype(np.float32).transpose(
            0, 2, 1
        )
    return out


# revision 10
# speedup vs baseline: 1.2157x; 1.2157x over previous
"""AFT-attention (nn_AFTAttention) distributed Bass kernel for 8 TRN2 NeuronCores.

Reference computation (B=8, T=4096, D=H=1024):
    Q = x @ Wq.T + bq ; K = x @ Wk.T + bk ; V = x @ Wv.T + bv      # [B,T,H]
    numer = exp(K + wbias)                                          # [B,T,H]
    denom = numer.sum(axis=0)                                       # [T,H]
    weighted = (numer * V).sum(axis=0) / denom                      # [T,H]
    out = sigmoid(Q) * weighted                                     # [B,T,H]

Sharding: the reductions are over the BATCH axis only, so sharding T across
the 8 cores makes every reduction core-local -- zero collectives.  Each core
handles all 8 batches for its 512-timestep slice.

Per-core layout: tiles are [h(128 partitions), t(512 free)] so every
per-feature constant (bq, bk+wbias, bv) rides the per-partition scalar port
of ScalarE activation / DVE scalar_tensor_tensor.  Host pre-transposes x to
x^T[b, d, t] and weights to W^T[d, h], and converts to bf16 (matmul runs at
1 col/cycle bf16 vs 4 cycles fp32).  sigmoid is computed via tanh (same ACT
table set as exp -> no 2.7us table reloads):
    sigmoid(q) * w == (tanh(q/2) + 1) * (w/2)
"""

import numpy as np

B, T, D, H = 8, 4096, 1024, 1024
NCORES = 8
TC = T // NCORES      # 512 timesteps per core
P = 128               # partitions
NCH = D // P          # 8 contraction chunks
NHT = H // P          # 8 h tiles
BG = 4                # batch group size for PSUM bank pressure

_cached = None        # (nc, run_fn)


def _build_bass():
    import concourse.bass as bass
    import concourse.mybir as mybir
    import concourse.tile as tile
    from concourse import bacc

    f32 = mybir.dt.float32
    bf16 = mybir.dt.bfloat16
    AF = mybir.ActivationFunctionType
    OP = mybir.AluOpType

    nc = bacc.Bacc(None)

    xt = nc.declare_dram_parameter("xt", [B, NCH, P, TC], bf16, isOutput=False)
    wk = nc.declare_dram_parameter("wk", [NCH, P, H], bf16, isOutput=False)
    wv = nc.declare_dram_parameter("wv", [NCH, P, H], bf16, isOutput=False)
    wq = nc.declare_dram_parameter("wq", [NCH, P, H], bf16, isOutput=False)
    # per-partition constants, host-prearranged as [P, NHT]
    bkw = nc.declare_dram_parameter("bkw", [P, NHT], f32, isOutput=False)  # bk + wbias
    bvp = nc.declare_dram_parameter("bvp", [P, NHT], f32, isOutput=False)  # bv
    bqh = nc.declare_dram_parameter("bqh", [P, NHT], f32, isOutput=False)  # 0.5*bq
    outt = nc.declare_dram_parameter("outt", [B, H, TC], bf16, isOutput=True)

    from contextlib import ExitStack

    with tile.TileContext(nc) as tc, ExitStack() as ctx:
        sing = ctx.enter_context(tc.tile_pool(name="sing", bufs=1))
        ps = ctx.enter_context(tc.tile_pool(name="ps", bufs=8, space="PSUM"))
        acc = ctx.enter_context(tc.tile_pool(name="acc", bufs=2))
        tmp = ctx.enter_context(tc.tile_pool(name="tmp", bufs=6))
        outp = ctx.enter_context(tc.tile_pool(name="outp", bufs=6))

        # --- resident inputs -------------------------------------------------
        # weights: [P, NCH, H] per matrix; lhsT slice = w_sb[:, c, j*P:(j+1)*P]
        wk_sb = sing.tile([P, NCH, H], bf16)
        wv_sb = sing.tile([P, NCH, H], bf16)
        wq_sb = sing.tile([P, NCH, H], bf16)
        # x^T: [P, B, NCH, TC]; rhs slice = xt_sb[:, b, c, :]
        xt_sb = sing.tile([P, B, NCH, TC], bf16)
        bkw_sb = sing.tile([P, NHT], f32)
        bvp_sb = sing.tile([P, NHT], f32)
        bqh_sb = sing.tile([P, NHT], f32)

        nc.sync.dma_start(out=bkw_sb, in_=bkw[:, :])
        nc.sync.dma_start(out=bvp_sb, in_=bvp[:, :])
        nc.sync.dma_start(out=bqh_sb, in_=bqh[:, :])
        # per-chunk DMAs, emitted in first-consumer order so the K matmuls of
        # (j=0, g=0) can start as soon as wk[c]+xt[b0..3][c] land
        for c in range(NCH):
            nc.sync.dma_start(out=wk_sb[:, c, :], in_=wk[c])
            for b in range(BG):
                nc.sync.dma_start(out=xt_sb[:, b, c, :], in_=xt[b, c])
        for c in range(NCH):
            nc.sync.dma_start(out=wv_sb[:, c, :], in_=wv[c])
        for c in range(NCH):
            for b in range(BG, B):
                nc.sync.dma_start(out=xt_sb[:, b, c, :], in_=xt[b, c])
        for c in range(NCH):
            nc.sync.dma_start(out=wq_sb[:, c, :], in_=wq[c])

        # --- main loop over h tiles -----------------------------------------
        for j in range(NHT):
            hs = bass.ts(j, P)  # h slice in weight free dim

            ns_sb = acc.tile([P, TC], f32, tag="ns")   # sum_b numer
            nv_sb = acc.tile([P, TC], f32, tag="nv")   # sum_b numer * v

            for g in range(B // BG):
                bs = range(g * BG, (g + 1) * BG)
                # K and V matmuls, chunk-outer / batch-inner (shares LDWEIGHTS)
                kps = {b: ps.tile([P, TC], f32, tag="ps", name=f"kps_{j}_{b}") for b in bs}
                vps = {b: ps.tile([P, TC], f32, tag="ps", name=f"vps_{j}_{b}") for b in bs}
                for c in range(NCH):
                    for b in bs:
                        nc.tensor.matmul(
                            kps[b], wk_sb[:, c, hs], xt_sb[:, b, c, :],
                            start=(c == 0), stop=(c == NCH - 1),
                        )
                    for b in bs:
                        nc.tensor.matmul(
                            vps[b], wv_sb[:, c, hs], xt_sb[:, b, c, :],
                            start=(c == 0), stop=(c == NCH - 1),
                        )
                # epilogue: numer = exp(k + bk + wbias); nv += numer*(v + bv)
                for b in bs:
                    if b == 0:
                        numer = ns_sb  # first batch: exp writes the accumulator
                    else:
                        numer = tmp.tile([P, TC], f32, tag="numer", bufs=4)
                    nc.scalar.activation(
                        out=numer, in_=kps[b], func=AF.Exp,
                        bias=bkw_sb[:, j : j + 1], scale=1.0,
                    )
                    if b == 0:
                        nvt = nv_sb
                    else:
                        nvt = tmp.tile([P, TC], f32, tag="nvt", bufs=3)
                    # nvt = (v_psum + bv) * numer
                    nc.vector.scalar_tensor_tensor(
                        out=nvt, in0=vps[b], scalar=bvp_sb[:, j : j + 1],
                        in1=numer, op0=OP.add, op1=OP.mult,
                    )
                    if b != 0:
                        nc.gpsimd.tensor_add(ns_sb, ns_sb, numer)
                        nc.vector.tensor_add(nv_sb, nv_sb, nvt)

            # weighted_half = 0.5 * nv / ns
            rec = tmp.tile([P, TC], f32, tag="rec", bufs=2)
            nc.vector.reciprocal(rec, ns_sb)
            wh = tmp.tile([P, TC], f32, tag="wh", bufs=2)
            nc.vector.scalar_tensor_tensor(
                out=wh, in0=nv_sb, scalar=0.5, in1=rec, op0=OP.mult, op1=OP.mult,
            )

            # Q matmuls + out = (tanh(q/2 + bq/2) + 1) * weighted_half
            for g in range(B // BG):
                bs = range(g * BG, (g + 1) * BG)
                qps = {b: ps.tile([P, TC], f32, tag="ps", name=f"qps_{j}_{b}") for b in bs}
                for c in range(NCH):
                    for b in bs:
                        nc.tensor.matmul(
                            qps[b], wq_sb[:, c, hs], xt_sb[:, b, c, :],
                            start=(c == 0), stop=(c == NCH - 1),
                        )
                for b in bs:
                    th = tmp.tile([P, TC], f32, tag="th", bufs=4)
                    nc.scalar.activation(
                        out=th, in_=qps[b], func=AF.Tanh,
                        bias=bqh_sb[:, j : j + 1], scale=0.5,
                    )
                    ot = outp.tile([P, TC], bf16, tag="ot")
                    nc.vector.scalar_tensor_tensor(
                        out=ot, in0=th, scalar=1.0, in1=wh, op0=OP.add, op1=OP.mult,
                    )
                    nc.sync.dma_start(out=outt[b, hs, :], in_=ot)

    nc.finalize()
    _dedup_ldweights(nc)
    return nc


def _dedup_ldweights(nc):
    """Drop InstLdweights that reload the exact weights already resident in
    the PE array (walrus's ldw-opt is disabled in this container, so every
    matmul otherwise gets its own LDWEIGHTS).  Sync carried by a removed
    LDWEIGHTS is preserved on an InstEventSemaphore in its place."""
    import concourse.mybir as mybir

    for bb in nc.m.functions[0].blocks:
        insts = list(bb.instructions)
        new = []
        prev_key = None
        changed = False
        for inst in insts:
            tname = type(inst).__name__
            if str(inst.engine) != "EngineType.PE":
                new.append(inst)
                continue
            if tname == "InstLdweights":
                key = (
                    str(inst.ins[0]),
                    str(inst.perf_mode),
                    str(inst.is_transpose),
                    str(inst.tile_position),
                )
                if key == prev_key:
                    si = inst.sync_info
                    if si is not None and (si.on_wait or si.on_update):
                        new.append(
                            mybir.InstEventSemaphore(
                                name=inst.name,
                                engine=inst.engine,
                                sync_info=si,
                                ins=[],
                                outs=[],
                            )
                        )
                    changed = True
                    continue
                prev_key = key
                new.append(inst)
            elif tname == "InstMatmult":
                new.append(inst)
            else:
                prev_key = None  # branches/drains: be conservative
                new.append(inst)
        if changed:
            del bb.instructions[:]
            for inst in new:
                bb.add_instruction(inst)


def _prepare_in_maps(x, Wq, bq, Wk, bk, Wv, bv, wbias):
    import ml_dtypes

    bf16 = ml_dtypes.bfloat16
    f32 = np.float32

    # weights: W.T [D, H] -> [NCH, P, H] bf16 (shared by all cores)
    def prep_w(w):
        return np.ascontiguousarray(w.T.astype(bf16)).reshape(NCH, P, H)

    wq_h = prep_w(np.asarray(Wq))
    wk_h = prep_w(np.asarray(Wk))
    wv_h = prep_w(np.asarray(Wv))

    # per-partition constants as [P, NHT]: col j holds values for h in [j*128, ...)
    def prep_b(v):
        return np.ascontiguousarray(np.asarray(v, f32).reshape(NHT, P).T)

    bkw_h = prep_b(np.asarray(bk, f32) + np.asarray(wbias, f32))
    bvp_h = prep_b(bv)
    bqh_h = prep_b(0.5 * np.asarray(bq, f32))

    x = np.asarray(x)
    in_maps = []
    for core in range(NCORES):
        xs = x[:, core * TC : (core + 1) * TC, :]           # [B, TC, D]
        xtc = np.ascontiguousarray(xs.transpose(0, 2, 1).astype(bf16)).reshape(
            B, NCH, P, TC
        )
        in_maps.append(
            {
                "xt": xtc,
                "wq": wq_h,
                "wk": wk_h,
                "wv": wv_h,
                "bkw": bkw_h,
                "bvp": bvp_h,
                "bqh": bqh_h,
            }
        )
    return in_maps


def _get_nc():
    global _cached
    if _cached is None:
        _cached = _build_bass()
    return _cached


TRACE = False          # set True from a test harness to profile
TRACE_TMPDIR = None    # optional persistent dir for trace artifacts
LAST_RESULT = None     # BassKernelResults of the most recent kernel() call


def kernel(x, Wq, bq, Wk, bk, Wv, bv, wbias):
    global LAST_RESULT
    from concourse.bass_utils import run_bass_kernel_spmd

    nc = _get_nc()
    in_maps = _prepare_in_maps(x, Wq, bq, Wk, bk, Wv, bv, wbias)
    kw = {}
    if TRACE:
        kw = {"trace": True, "tmpdir": TRACE_TMPDIR}
    res = run_bass_kernel_spmd(nc, in_maps, core_ids=list(range(NCORES)), **kw)
    LAST_RESULT = res
    out = np.empty((B, T, H), np.float32)
    for core in range(NCORES):
        o = np.asarray(res.results[core]["outt"])            # [B, H, TC] bf16
        out[:, core * TC : (core + 1) * TC, :] = o.astype(np.float32).transpose(
            0, 2, 1
        )
    return out
